# revision 5
# baseline (speedup 1.0000x reference)
"""Trainium2 Bass kernel for nn_Attention (B=4, L=1024, D=768, H=12, DH=64).

Reference computation per (batch b, head n):
    K = k_n @ x_b^T            [D, L]
    Q = q_n @ x_b^T            [D, L]
    scores = Q^T K             [L, L]   (contraction over D=768)
    S = softmax(scores, -1)
    V = v_n @ x_b^T            [DH, L]
    R = V @ S^T                [DH, L]
    out[b, l, n*DH+e] = R[e, l]

Sharding: 48 independent (b, n) units over 8 cores -> core c owns batch
b = c//2 and heads 6*(c%2) .. 6*(c%2)+6.  No collectives; host does the
final gather/transpose.

Device math per head (A-trick): scores = x_b (q_n^T k_n) x_b^T
    A  = q_n^T k_n             [D, D]     (fp32r matmuls, full PE rate)
    WT = A^T x_b^T             [D, L]
    scores_block = WT^T x_b^T  [128, L]   per 128-row block of l
    S' = exp(scores)           (no max subtraction: logits are O(1))
    P = S' / rowsum            (bf16)
    R[e, l] += (P^T block)-matmuls against VT in bf16
Output per core: out_r [384, 1024] = R stacked over 6 heads; host
transposes into out[b, :, 384g:384g+384].

fp32r: hardware requires fp32r matmul operands to be produced by an
explicit rounding op into their memory location, so DMA'd f32 goes
through a staging tile + tensor_copy into an f32r tile; PSUM->SBUF
copies write f32r directly.
"""

from contextlib import ExitStack

import numpy as np

import concourse.bass as bass
import concourse.tile as tile
from concourse import bacc, mybir
from concourse.bass import ts, ds
from concourse.bass_utils import run_bass_kernel_spmd
from concourse.masks import make_identity

B, L, D, H = 4, 1024, 768, 12
DH = D // H          # 64
HPC = 6              # heads per core
N_CORES = 8
DC = D // 128        # 6 chunks of the contraction/feature dim
LB = L // 128        # 8 l-blocks
F32 = mybir.dt.float32
F32R = mybir.dt.float32r
BF16 = mybir.dt.bfloat16

_COMPILED = None


def _build():
    nc = bacc.Bacc(
        "TRN2",
        target_bir_lowering=False,
        debug=False,
        enable_asserts=False,
        num_devices=N_CORES,
    )
    xT_ext = nc.dram_tensor("xT", [D, L], F32, kind="ExternalInput").ap()
    q6_ext = nc.dram_tensor("q6", [HPC, D, D], F32, kind="ExternalInput").ap()
    k6_ext = nc.dram_tensor("k6", [HPC, D, D], F32, kind="ExternalInput").ap()
    vT6_ext = nc.dram_tensor("vT6", [D, HPC * DH], F32, kind="ExternalInput").ap()
    out_ext = nc.dram_tensor("out_r", [HPC * DH, L], F32, kind="ExternalOutput").ap()

    with tile.TileContext(nc) as tc, ExitStack() as ctx:
        const_pool = ctx.enter_context(tc.tile_pool(name="const", bufs=1))
        stage_pool = ctx.enter_context(tc.tile_pool(name="stage", bufs=3))
        xt_pool = ctx.enter_context(tc.tile_pool(name="xt", bufs=1))
        vt_pool = ctx.enter_context(tc.tile_pool(name="vt", bufs=1))
        ps_proj = ctx.enter_context(tc.tile_pool(name="ps_proj", bufs=2, space="PSUM"))
        ps_s = ctx.enter_context(tc.tile_pool(name="ps_s", bufs=2, space="PSUM"))
        ps_t = ctx.enter_context(tc.tile_pool(name="ps_t", bufs=2, space="PSUM"))

        ident = const_pool.tile([128, 128], BF16)
        make_identity(nc, ident[:])

        # ---- load + round xT (6 chunks of [128, L]) ----
        xt = []
        for i in range(DC):
            stg = stage_pool.tile([128, L], F32, tag="stg")
            nc.sync.dma_start(stg[:], xT_ext[ts(i, 128), :])
            t = xt_pool.tile([128, L], F32R, tag=f"xt{i}")
            nc.vector.tensor_copy(t[:], stg[:])
            xt.append(t)

        # ---- VT_all[m, he] = sum_d xT[d,m] * vT6[d,he]  (bf16 out) ----
        vt = []
        with tc.tile_pool(name="vt6", bufs=1) as vt6_pool:
            vt6 = []
            for i in range(DC):
                stg = stage_pool.tile([128, L], F32, tag="stg")
                nc.sync.dma_start(stg[:, : HPC * DH], vT6_ext[ts(i, 128), :])
                t = vt6_pool.tile([128, HPC * DH], F32R, tag=f"vt6_{i}")
                nc.vector.tensor_copy(t[:], stg[:, : HPC * DH])
                vt6.append(t)
            for j in range(LB):
                p = ps_proj.tile([128, 512], F32, tag="ps_p")
                for i in range(DC):
                    nc.tensor.matmul(
                        p[:, : HPC * DH],
                        xt[i][:, ts(j, 128)],
                        vt6[i][:],
                        start=(i == 0),
                        stop=(i == DC - 1),
                    )
                t = vt_pool.tile([128, HPC * DH], BF16, tag=f"vt{j}")
                nc.vector.tensor_copy(t[:], p[:, : HPC * DH])
                vt.append(t)

        # per-head working pools (created after vt6 released to fit SBUF)
        qk_pool = ctx.enter_context(tc.tile_pool(name="qk", bufs=2))
        a_pool = ctx.enter_context(tc.tile_pool(name="a", bufs=1))
        wt_pool = ctx.enter_context(tc.tile_pool(name="wt", bufs=1))
        soft_pool = ctx.enter_context(tc.tile_pool(name="soft", bufs=2))
        st_pool = ctx.enter_context(tc.tile_pool(name="st", bufs=2))
        out_pool = ctx.enter_context(tc.tile_pool(name="outp", bufs=2))

        for h in range(HPC):
            # ---- load + round q_h, k_h ----
            q_sb, k_sb = [], []
            for i in range(DC):
                stq = stage_pool.tile([128, L], F32, tag="stg")
                nc.sync.dma_start(stq[:, :D], q6_ext[h, ts(i, 128), :])
                tq = qk_pool.tile([128, D], F32R, tag=f"q{i}")
                nc.scalar.copy(tq[:], stq[:, :D])
                q_sb.append(tq)
                stk = stage_pool.tile([128, L], F32, tag="stg")
                nc.sync.dma_start(stk[:, :D], k6_ext[h, ts(i, 128), :])
                tk = qk_pool.tile([128, D], F32R, tag=f"k{i}")
                nc.scalar.copy(tk[:], stk[:, :D])
                k_sb.append(tk)

            # ---- A[d, d'] = sum_c q[c,d] k[c,d'] ----
            a_sb = []
            for i in range(DC):
                t = a_pool.tile([128, D], F32R, tag=f"a{i}")
                for n in range(2):
                    p = ps_proj.tile([128, 512], F32, tag="ps_p")
                    for j in range(DC):
                        nc.tensor.matmul(
                            p[:, :384],
                            q_sb[j][:, ts(i, 128)],
                            k_sb[j][:, ts(n, 384)],
                            start=(j == 0),
                            stop=(j == DC - 1),
                        )
                    nc.vector.tensor_copy(t[:, ts(n, 384)], p[:, :384])
                a_sb.append(t)

            # ---- WT[d', l] = sum_d A[d,d'] xT[d,l] ----
            wt_sb = []
            for i in range(DC):
                t = wt_pool.tile([128, L], F32R, tag=f"wt{i}")
                for n in range(2):
                    p = ps_proj.tile([128, 512], F32, tag="ps_p")
                    for j in range(DC):
                        nc.tensor.matmul(
                            p[:],
                            a_sb[j][:, ts(i, 128)],
                            xt[j][:, ts(n, 512)],
                            start=(j == 0),
                            stop=(j == DC - 1),
                        )
                    nc.vector.tensor_copy(t[:, ts(n, 512)], p[:])
                wt_sb.append(t)

            # ---- l-blocks: scores -> softmax -> transpose -> R ----
            out_sb = out_pool.tile([DH, L], F32, tag="out")

            def scores(li):
                p = ps_s.tile([128, L], F32, tag="ps_s")
                for n in range(2):
                    for j in range(DC):
                        nc.tensor.matmul(
                            p[:, ts(n, 512)],
                            wt_sb[j][:, ts(li, 128)],
                            xt[j][:, ts(n, 512)],
                            start=(j == 0),
                            stop=(j == DC - 1),
                        )
                return p

            ps_prev = scores(0)
            for li in range(LB):
                ps_cur = ps_prev
                if li + 1 < LB:
                    ps_prev = scores(li + 1)
                # softmax over free dim (no max subtraction needed)
                s_f32 = soft_pool.tile([128, L], F32, tag="s")
                sums = soft_pool.tile([128, 1], F32, tag="sums")
                nc.scalar.activation(
                    s_f32[:],
                    ps_cur[:],
                    mybir.ActivationFunctionType.Exp,
                    accum_out=sums[:],
                )
                recip = soft_pool.tile([128, 1], F32, tag="recip")
                nc.vector.reciprocal(recip[:], sums[:])
                p_bf = soft_pool.tile([128, L], BF16, tag="p")
                nc.vector.tensor_scalar_mul(p_bf[:], s_f32[:], recip[:])

                # transpose P 128x128 blocks, then R[e, l] += VT^T P^T
                st_sb = []
                for mj in range(LB):
                    pt = ps_t.tile([128, 128], BF16, tag="ps_t")
                    nc.tensor.transpose(pt[:], p_bf[:, ts(mj, 128)], ident[:])
                    t = st_pool.tile([128, 128], BF16, tag=f"st{mj}")
                    nc.scalar.copy(t[:], pt[:])
                    st_sb.append(t)
                pr = ps_proj.tile([DH, 128], F32, tag="ps_p")
                for mj in range(LB):
                    nc.tensor.matmul(
                        pr[:],
                        vt[mj][:, ts(h, DH)],
                        st_sb[mj][:],
                        start=(mj == 0),
                        stop=(mj == LB - 1),
                    )
                nc.vector.tensor_copy(out_sb[:, ts(li, 128)], pr[:])

            nc.sync.dma_start(out_ext[ts(h, DH), :], out_sb[:])

    nc.compile()
    return nc


def kernel(x, k, q, v):
    global _COMPILED
    if _COMPILED is None:
        _COMPILED = _build()

    x = np.ascontiguousarray(x, dtype=np.float32)
    k = np.ascontiguousarray(k, dtype=np.float32)
    q = np.ascontiguousarray(q, dtype=np.float32)
    v = np.ascontiguousarray(v, dtype=np.float32)

    in_maps = []
    for c in range(N_CORES):
        b, g = c // 2, c % 2
        hs = slice(HPC * g, HPC * (g + 1))
        in_maps.append(
            {
                "xT": np.ascontiguousarray(x[b].T),
                "q6": q[hs],
                "k6": k[hs],
                "vT6": np.ascontiguousarray(
                    v[hs].transpose(2, 0, 1).reshape(D, HPC * DH)
                ),
            }
        )

    res = run_bass_kernel_spmd(_COMPILED, in_maps, core_ids=list(range(N_CORES)))

    out = np.empty((B, L, D), np.float32)
    for c in range(N_CORES):
        b, g = c // 2, c % 2
        out[b, :, HPC * DH * g : HPC * DH * (g + 1)] = res.results[c]["out_r"].T
    return out


if __name__ == "__main__":
    rng = np.random.default_rng(0)
    x = rng.standard_normal((B, L, D)).astype(np.float32)
    k = (rng.random((H, D, D)) / D).astype(np.float32)
    q = (rng.random((H, D, D)) / D).astype(np.float32)
    v = (rng.random((H, DH, D)) / D).astype(np.float32)
    o = kernel(x=x, k=k, q=q, v=v)
    print("out", o.shape, o.dtype)


# revision 7
# speedup vs baseline: 1.5743x; 1.5743x over previous
"""Trainium2 Bass kernel for nn_Attention (B=4, L=1024, D=768, H=12, DH=64).

Reference per (batch b, head n):
    K = k_n @ x_b^T; Q = q_n @ x_b^T        [D, L]
    scores = Q^T K                          [L, L]
    S = softmax(scores, -1)
    V = v_n @ x_b^T                         [DH, L]
    out[b, l, n*DH+e] = sum_m S[l, m] V[e, m]

Sharding: 48 independent (b, n) units over 8 cores -> core c owns batch
b = c//2 and heads 6*(c%2)..+6.  No collectives; host only slices
inputs and concatenates outputs.

Device math per head (A-trick, all bf16 matmuls / f32 PSUM):
    A  = q_n^T k_n                [D, D]
    WT = A^T x_b^T                [D, L]
    sT(mj) = xtT-block^T @ WT     [128m, L]   (scores transposed)
    pT(mj) = exp(sT(mj))          bf16        (no max subtraction:
                                               logits are O(1) here)
    R^T[l-block, 0:64] + sums[l] via matmul with vt_aug (V^T plus a
    ones column -> column 64 accumulates sum_m exp) accumulated over mj
    out_block = R^T * (1/sums)    per-partition tensor_scalar
Output per core: out_r [1024, 384] with columns 64h..64h+64 = head h;
host writes it straight into out[b, :, 384g:384g+384].
"""

from contextlib import ExitStack

import numpy as np

import concourse.bass as bass
import concourse.tile as tile
from concourse import bacc, mybir
from concourse.bass import ts, ds
from concourse.bass_utils import run_bass_kernel_spmd

B, L, D, H = 4, 1024, 768, 12
DH = D // H          # 64
HPC = 6              # heads per core
N_CORES = 8
DC = D // 128        # 6 chunks of the contraction/feature dim
LB = L // 128        # 8 l-blocks / m-blocks
DHA = DH + 1         # 65: head slice width in vt_aug (ones column at 64)
F32 = mybir.dt.float32
BF16 = mybir.dt.bfloat16

_COMPILED = None


def _build():
    nc = bacc.Bacc(
        "TRN2",
        target_bir_lowering=False,
        debug=False,
        enable_asserts=False,
        num_devices=N_CORES,
    )
    xT_ext = nc.dram_tensor("xT", [D, L], F32, kind="ExternalInput").ap()
    q6_ext = nc.dram_tensor("q6", [HPC, D, D], F32, kind="ExternalInput").ap()
    k6_ext = nc.dram_tensor("k6", [HPC, D, D], F32, kind="ExternalInput").ap()
    vT6_ext = nc.dram_tensor("vT6", [D, HPC * DH], F32, kind="ExternalInput").ap()
    out_ext = nc.dram_tensor("out_r", [L, HPC * DH], F32, kind="ExternalOutput").ap()

    with tile.TileContext(nc) as tc, ExitStack() as ctx:
        stage_pool = ctx.enter_context(tc.tile_pool(name="stage", bufs=3))
        xt_pool = ctx.enter_context(tc.tile_pool(name="xt", bufs=1))
        vt_pool = ctx.enter_context(tc.tile_pool(name="vt", bufs=1))
        qk_pool = ctx.enter_context(tc.tile_pool(name="qk", bufs=2))
        a_pool = ctx.enter_context(tc.tile_pool(name="a", bufs=1))
        wt_pool = ctx.enter_context(tc.tile_pool(name="wt", bufs=1))
        pt_pool = ctx.enter_context(tc.tile_pool(name="pt", bufs=1))
        soft_pool = ctx.enter_context(tc.tile_pool(name="soft", bufs=2))
        out_pool = ctx.enter_context(tc.tile_pool(name="outp", bufs=1))
        ps_p = ctx.enter_context(tc.tile_pool(name="ps_p", bufs=2, space="PSUM"))
        ps_s = ctx.enter_context(tc.tile_pool(name="ps_s", bufs=2, space="PSUM"))
        ps_r = ctx.enter_context(tc.tile_pool(name="ps_r", bufs=2, space="PSUM"))

        # ---- load + cast xT to bf16 (6 chunks of [128, L]) ----
        xt = []
        for i in range(DC):
            stg = stage_pool.tile([128, L], F32, tag="stg")
            nc.sync.dma_start(stg[:], xT_ext[ts(i, 128), :])
            t = xt_pool.tile([128, L], BF16, tag=f"xt{i}")
            nc.vector.tensor_copy(t[:], stg[:])
            xt.append(t)

        # ---- VT_aug[mj]: [128m, 390] = per-head 64 cols of V^T + ones col ----
        vt = []
        with tc.tile_pool(name="vt6", bufs=1) as vt6_pool:
            vt6 = []
            for i in range(DC):
                stg = stage_pool.tile([128, L], F32, tag="stg")
                nc.sync.dma_start(stg[:, : HPC * DH], vT6_ext[ts(i, 128), :])
                t = vt6_pool.tile([128, HPC * DH], BF16, tag=f"vt6_{i}")
                nc.vector.tensor_copy(t[:], stg[:, : HPC * DH])
                vt6.append(t)
            for j in range(LB):
                p = ps_p.tile([128, 512], F32, tag="ps_p")
                for i in range(DC):
                    nc.tensor.matmul(
                        p[:, : HPC * DH],
                        xt[i][:, ts(j, 128)],
                        vt6[i][:],
                        start=(i == 0),
                        stop=(i == DC - 1),
                    )
                t = vt_pool.tile([128, HPC * DHA], BF16, tag=f"vt{j}")
                nc.gpsimd.memset(t[:], 1.0)
                t3 = t[:].rearrange("p (h c) -> p h c", h=HPC)
                p3 = p[:, : HPC * DH].rearrange("p (h c) -> p h c", h=HPC)
                nc.vector.tensor_copy(t3[:, :, :DH], p3[:])
                vt.append(t)

        # out accumulators: one [128, 384] f32 tile per l-block, all heads
        out_sb = []
        for lb in range(LB):
            ot = out_pool.tile([128, HPC * DH], F32, tag=f"out{lb}")
            out_sb.append(ot)

        for h in range(HPC):
            # ---- load + cast q_h, k_h ----
            q_sb, k_sb = [], []
            for i in range(DC):
                stq = stage_pool.tile([128, L], F32, tag="stg")
                nc.sync.dma_start(stq[:, :D], q6_ext[h, ts(i, 128), :])
                tq = qk_pool.tile([128, D], BF16, tag=f"q{i}")
                nc.scalar.copy(tq[:], stq[:, :D])
                q_sb.append(tq)
                stk = stage_pool.tile([128, L], F32, tag="stg")
                nc.sync.dma_start(stk[:, :D], k6_ext[h, ts(i, 128), :])
                tk = qk_pool.tile([128, D], BF16, tag=f"k{i}")
                nc.scalar.copy(tk[:], stk[:, :D])
                k_sb.append(tk)

            # ---- A[d, d'] = sum_c q[c,d] k[c,d'] ----
            a_sb = []
            for i in range(DC):
                t = a_pool.tile([128, D], BF16, tag=f"a{i}")
                for n in range(2):
                    p = ps_p.tile([128, 512], F32, tag="ps_p")
                    for j in range(DC):
                        nc.tensor.matmul(
                            p[:, :384],
                            q_sb[j][:, ts(i, 128)],
                            k_sb[j][:, ts(n, 384)],
                            start=(j == 0),
                            stop=(j == DC - 1),
                        )
                    nc.vector.tensor_copy(t[:, ts(n, 384)], p[:, :384])
                a_sb.append(t)

            # ---- WT[d', l] = sum_d A[d,d'] xT[d,l] ----
            wt_sb = []
            for i in range(DC):
                t = wt_pool.tile([128, L], BF16, tag=f"wt{i}")
                for n in range(2):
                    p = ps_p.tile([128, 512], F32, tag="ps_p")
                    for j in range(DC):
                        nc.tensor.matmul(
                            p[:],
                            a_sb[j][:, ts(i, 128)],
                            xt[j][:, ts(n, 512)],
                            start=(j == 0),
                            stop=(j == DC - 1),
                        )
                    nc.vector.tensor_copy(t[:, ts(n, 512)], p[:])
                wt_sb.append(t)

            # ---- scoresT blocks + exp (pipelined over mj) ----
            def scores_t(mj):
                p = ps_s.tile([128, L], F32, tag="ps_s")
                for n in range(2):
                    for j in range(DC):
                        nc.tensor.matmul(
                            p[:, ts(n, 512)],
                            xt[j][:, ts(mj, 128)],
                            wt_sb[j][:, ts(n, 512)],
                            start=(j == 0),
                            stop=(j == DC - 1),
                        )
                return p

            pt_sb = []
            ps_prev = scores_t(0)
            for mj in range(LB):
                ps_cur = ps_prev
                if mj + 1 < LB:
                    ps_prev = scores_t(mj + 1)
                pt = pt_pool.tile([128, L], BF16, tag=f"pt{mj}")
                nc.scalar.activation(
                    pt[:], ps_cur[:], mybir.ActivationFunctionType.Exp
                )
                pt_sb.append(pt)

            # ---- R^T per l-block: [128l, 65] = sum_mj pT[mj][:,lb]^T @ vt_aug
            #      column 64 = sum_m exp  -> per-partition normalize ----
            for lb in range(LB):
                pr = ps_r.tile([128, DHA], F32, tag="ps_r")
                for mj in range(LB):
                    nc.tensor.matmul(
                        pr[:],
                        pt_sb[mj][:, ts(lb, 128)],
                        vt[mj][:, ds(DHA * h, DHA)],
                        start=(mj == 0),
                        stop=(mj == LB - 1),
                    )
                recip = soft_pool.tile([128, 1], F32, tag="recip")
                nc.vector.reciprocal(recip[:], pr[:, DH : DH + 1])
                nc.vector.tensor_scalar_mul(
                    out_sb[lb][:, ts(h, DH)], pr[:, :DH], recip[:]
                )

        for lb in range(LB):
            nc.sync.dma_start(out_ext[ts(lb, 128), :], out_sb[lb][:])

    nc.compile()
    return nc


def kernel(x, k, q, v):
    global _COMPILED
    if _COMPILED is None:
        _COMPILED = _build()

    x = np.ascontiguousarray(x, dtype=np.float32)
    k = np.ascontiguousarray(k, dtype=np.float32)
    q = np.ascontiguousarray(q, dtype=np.float32)
    v = np.ascontiguousarray(v, dtype=np.float32)

    in_maps = []
    for c in range(N_CORES):
        b, g = c // 2, c % 2
        hs = slice(HPC * g, HPC * (g + 1))
        in_maps.append(
            {
                "xT": np.ascontiguousarray(x[b].T),
                "q6": q[hs],
                "k6": k[hs],
                "vT6": np.ascontiguousarray(
                    v[hs].transpose(2, 0, 1).reshape(D, HPC * DH)
                ),
            }
        )

    res = run_bass_kernel_spmd(_COMPILED, in_maps, core_ids=list(range(N_CORES)))

    out = np.empty((B, L, D), np.float32)
    for c in range(N_CORES):
        b, g = c // 2, c % 2
        out[b, :, HPC * DH * g : HPC * DH * (g + 1)] = res.results[c]["out_r"]
    return out


if __name__ == "__main__":
    rng = np.random.default_rng(0)
    x = rng.standard_normal((B, L, D)).astype(np.float32)
    k = (rng.random((H, D, D)) / D).astype(np.float32)
    q = (rng.random((H, D, D)) / D).astype(np.float32)
    v = (rng.random((H, DH, D)) / D).astype(np.float32)
    o = kernel(x=x, k=k, q=q, v=v)
    print("out", o.shape, o.dtype)


# revision 8
# speedup vs baseline: 1.6511x; 1.0488x over previous
"""Trainium2 Bass kernel for nn_Attention (B=4, L=1024, D=768, H=12, DH=64).

Reference per (batch b, head n):
    K = k_n @ x_b^T; Q = q_n @ x_b^T        [D, L]
    scores = Q^T K                          [L, L]
    S = softmax(scores, -1)
    V = v_n @ x_b^T                         [DH, L]
    out[b, l, n*DH+e] = sum_m S[l, m] V[e, m]

Sharding: 48 independent (b, n) units over 8 cores.  Core c owns the
batch PAIR bp = c//4 (batches 2bp, 2bp+1) and the head TRIPLE t = c%4
(heads 3t..3t+3): A = q^T k is computed once per head and reused for
both batches.  No collectives; host only slices inputs and
concatenates outputs.

Device math per (head, batch) with all-bf16 matmuls / f32 PSUM:
    A  = q_n^T k_n                [D, D]    (once per head)
    WT = A^T x_b^T                [D, L]
    sT(mj) = xtT-block^T @ WT     [128m, L]  (scores transposed)
    pT(mj) = exp(sT(mj))          bf16       (no max subtraction:
                                              logits are O(1) here)
    R^T[l-block, 0:64] + sums[l] via matmul with vt_aug (V^T plus a
    ones column -> column 64 accumulates sum_m exp) accumulated over mj
    out_block = R^T * (1/sums)    per-partition tensor_scalar
Output per core: out_r [2, L, 192]; host writes out_r[bi] straight
into out[2bp+bi, :, 192t:192t+192].
"""

from contextlib import ExitStack

import numpy as np

import concourse.bass as bass
import concourse.tile as tile
from concourse import bacc, mybir
from concourse.bass import ts, ds
from concourse.bass_utils import run_bass_kernel_spmd

B, L, D, H = 4, 1024, 768, 12
DH = D // H          # 64
HPC = 3              # heads per core
BPC = 2              # batches per core
N_CORES = 8
DC = D // 128        # 6 chunks of the contraction/feature dim
LB = L // 128        # 8 l-blocks / m-blocks
DHA = DH + 1         # 65: head slice width in vt_aug (ones column at 64)
F32 = mybir.dt.float32
BF16 = mybir.dt.bfloat16

_COMPILED = None


def _build():
    nc = bacc.Bacc(
        "TRN2",
        target_bir_lowering=False,
        debug=False,
        enable_asserts=False,
        num_devices=N_CORES,
    )
    xT_ext = nc.dram_tensor("xT", [BPC, D, L], F32, kind="ExternalInput").ap()
    q3_ext = nc.dram_tensor("q3", [HPC, D, D], F32, kind="ExternalInput").ap()
    k3_ext = nc.dram_tensor("k3", [HPC, D, D], F32, kind="ExternalInput").ap()
    vT3_ext = nc.dram_tensor("vT3", [D, HPC * DH], F32, kind="ExternalInput").ap()
    out_ext = nc.dram_tensor(
        "out_r", [BPC, L, HPC * DH], F32, kind="ExternalOutput"
    ).ap()

    with tile.TileContext(nc) as tc, ExitStack() as ctx:
        stage_pool = ctx.enter_context(tc.tile_pool(name="stage", bufs=3))
        xt_pool = ctx.enter_context(tc.tile_pool(name="xt", bufs=1))
        vt_pool = ctx.enter_context(tc.tile_pool(name="vt", bufs=1))
        qk_pool = ctx.enter_context(tc.tile_pool(name="qk", bufs=2))
        a_pool = ctx.enter_context(tc.tile_pool(name="a", bufs=1))
        wt_pool = ctx.enter_context(tc.tile_pool(name="wt", bufs=1))
        pt_pool = ctx.enter_context(tc.tile_pool(name="pt", bufs=1))
        soft_pool = ctx.enter_context(tc.tile_pool(name="soft", bufs=2))
        out_pool = ctx.enter_context(tc.tile_pool(name="outp", bufs=1))
        ps_p = ctx.enter_context(tc.tile_pool(name="ps_p", bufs=2, space="PSUM"))
        ps_s = ctx.enter_context(tc.tile_pool(name="ps_s", bufs=2, space="PSUM"))
        ps_r = ctx.enter_context(tc.tile_pool(name="ps_r", bufs=2, space="PSUM"))

        # ---- load + cast xT to bf16 (2 batches x 6 chunks of [128, L]) ----
        xt = [[], []]
        for bi in range(BPC):
            for i in range(DC):
                stg = stage_pool.tile([128, L], F32, tag="stg")
                nc.sync.dma_start(stg[:], xT_ext[bi, ts(i, 128), :])
                t = xt_pool.tile([128, L], BF16, tag=f"xt{bi}_{i}")
                nc.vector.tensor_copy(t[:], stg[:])
                xt[bi].append(t)

        # ---- VT_aug[bi][mj]: [128m, 195] = per-head V^T cols + ones col ----
        vt = [[], []]
        with tc.tile_pool(name="vt3", bufs=1) as vt3_pool:
            vt3 = []
            for i in range(DC):
                stg = stage_pool.tile([128, L], F32, tag="stg")
                nc.sync.dma_start(stg[:, : HPC * DH], vT3_ext[ts(i, 128), :])
                t = vt3_pool.tile([128, HPC * DH], BF16, tag=f"vt3_{i}")
                nc.vector.tensor_copy(t[:], stg[:, : HPC * DH])
                vt3.append(t)
            for bi in range(BPC):
                for j in range(LB):
                    p = ps_p.tile([128, 512], F32, tag="ps_p")
                    for i in range(DC):
                        nc.tensor.matmul(
                            p[:, : HPC * DH],
                            xt[bi][i][:, ts(j, 128)],
                            vt3[i][:],
                            start=(i == 0),
                            stop=(i == DC - 1),
                        )
                    t = vt_pool.tile([128, HPC * DHA], BF16, tag=f"vt{bi}_{j}")
                    nc.gpsimd.memset(t[:], 1.0)
                    t3 = t[:].rearrange("p (h c) -> p h c", h=HPC)
                    p3 = p[:, : HPC * DH].rearrange("p (h c) -> p h c", h=HPC)
                    nc.vector.tensor_copy(t3[:, :, :DH], p3[:])
                    vt[bi].append(t)

        # out accumulators: per batch, one [128, 192] f32 tile per l-block
        out_sb = [[], []]
        for bi in range(BPC):
            for lb in range(LB):
                ot = out_pool.tile([128, HPC * DH], F32, tag=f"out{bi}_{lb}")
                out_sb[bi].append(ot)

        for h in range(HPC):
            # ---- load + cast q_h, k_h ----
            q_sb, k_sb = [], []
            for i in range(DC):
                stq = stage_pool.tile([128, L], F32, tag="stg")
                nc.sync.dma_start(stq[:, :D], q3_ext[h, ts(i, 128), :])
                tq = qk_pool.tile([128, D], BF16, tag=f"q{i}")
                nc.scalar.copy(tq[:], stq[:, :D])
                q_sb.append(tq)
                stk = stage_pool.tile([128, L], F32, tag="stg")
                nc.sync.dma_start(stk[:, :D], k3_ext[h, ts(i, 128), :])
                tk = qk_pool.tile([128, D], BF16, tag=f"k{i}")
                nc.scalar.copy(tk[:], stk[:, :D])
                k_sb.append(tk)

            # ---- A[d, d'] = sum_c q[c,d] k[c,d']  (once per head) ----
            a_sb = []
            for i in range(DC):
                t = a_pool.tile([128, D], BF16, tag=f"a{i}")
                for n in range(2):
                    p = ps_p.tile([128, 512], F32, tag="ps_p")
                    for j in range(DC):
                        nc.tensor.matmul(
                            p[:, :384],
                            q_sb[j][:, ts(i, 128)],
                            k_sb[j][:, ts(n, 384)],
                            start=(j == 0),
                            stop=(j == DC - 1),
                        )
                    nc.vector.tensor_copy(t[:, ts(n, 384)], p[:, :384])
                a_sb.append(t)

            for bi in range(BPC):
                xtb = xt[bi]
                # ---- WT[d', l] = sum_d A[d,d'] xT[d,l] ----
                wt_sb = []
                for i in range(DC):
                    t = wt_pool.tile([128, L], BF16, tag=f"wt{i}")
                    for n in range(2):
                        p = ps_p.tile([128, 512], F32, tag="ps_p")
                        for j in range(DC):
                            nc.tensor.matmul(
                                p[:],
                                a_sb[j][:, ts(i, 128)],
                                xtb[j][:, ts(n, 512)],
                                start=(j == 0),
                                stop=(j == DC - 1),
                            )
                        nc.vector.tensor_copy(t[:, ts(n, 512)], p[:])
                    wt_sb.append(t)

                # ---- scoresT blocks + exp (pipelined over mj) ----
                def scores_t(mj):
                    p = ps_s.tile([128, L], F32, tag="ps_s")
                    for n in range(2):
                        for j in range(DC):
                            nc.tensor.matmul(
                                p[:, ts(n, 512)],
                                xtb[j][:, ts(mj, 128)],
                                wt_sb[j][:, ts(n, 512)],
                                start=(j == 0),
                                stop=(j == DC - 1),
                            )
                    return p

                pt_sb = []
                ps_prev = scores_t(0)
                for mj in range(LB):
                    ps_cur = ps_prev
                    if mj + 1 < LB:
                        ps_prev = scores_t(mj + 1)
                    pt = pt_pool.tile([128, L], BF16, tag=f"pt{mj}")
                    nc.scalar.activation(
                        pt[:], ps_cur[:], mybir.ActivationFunctionType.Exp
                    )
                    pt_sb.append(pt)

                # ---- R^T per l-block + fused sums -> normalize ----
                for lb in range(LB):
                    pr = ps_r.tile([128, DHA], F32, tag="ps_r")
                    for mj in range(LB):
                        nc.tensor.matmul(
                            pr[:],
                            pt_sb[mj][:, ts(lb, 128)],
                            vt[bi][mj][:, ds(DHA * h, DHA)],
                            start=(mj == 0),
                            stop=(mj == LB - 1),
                        )
                    recip = soft_pool.tile([128, 1], F32, tag="recip")
                    nc.vector.reciprocal(recip[:], pr[:, DH : DH + 1])
                    nc.vector.tensor_scalar_mul(
                        out_sb[bi][lb][:, ts(h, DH)], pr[:, :DH], recip[:]
                    )

        for bi in range(BPC):
            for lb in range(LB):
                nc.sync.dma_start(out_ext[bi, ts(lb, 128), :], out_sb[bi][lb][:])

    nc.compile()
    return nc


def kernel(x, k, q, v):
    global _COMPILED
    if _COMPILED is None:
        _COMPILED = _build()

    x = np.ascontiguousarray(x, dtype=np.float32)
    k = np.ascontiguousarray(k, dtype=np.float32)
    q = np.ascontiguousarray(q, dtype=np.float32)
    v = np.ascontiguousarray(v, dtype=np.float32)

    in_maps = []
    for c in range(N_CORES):
        bp, t = c // 4, c % 4
        hs = slice(HPC * t, HPC * (t + 1))
        in_maps.append(
            {
                "xT": np.ascontiguousarray(
                    x[BPC * bp : BPC * (bp + 1)].transpose(0, 2, 1)
                ),
                "q3": q[hs],
                "k3": k[hs],
                "vT3": np.ascontiguousarray(
                    v[hs].transpose(2, 0, 1).reshape(D, HPC * DH)
                ),
            }
        )

    res = run_bass_kernel_spmd(_COMPILED, in_maps, core_ids=list(range(N_CORES)))

    out = np.empty((B, L, D), np.float32)
    for c in range(N_CORES):
        bp, t = c // 4, c % 4
        for bi in range(BPC):
            out[BPC * bp + bi, :, HPC * DH * t : HPC * DH * (t + 1)] = res.results[
                c
            ]["out_r"][bi]
    return out


if __name__ == "__main__":
    rng = np.random.default_rng(0)
    x = rng.standard_normal((B, L, D)).astype(np.float32)
    k = (rng.random((H, D, D)) / D).astype(np.float32)
    q = (rng.random((H, D, D)) / D).astype(np.float32)
    v = (rng.random((H, DH, D)) / D).astype(np.float32)
    o = kernel(x=x, k=k, q=q, v=v)
    print("out", o.shape, o.dtype)


# revision 9
# speedup vs baseline: 1.8356x; 1.1117x over previous
"""Trainium2 Bass kernel for nn_Attention (B=4, L=1024, D=768, H=12, DH=64).

Reference per (batch b, head n):
    K = k_n @ x_b^T; Q = q_n @ x_b^T        [D, L]
    scores = Q^T K                          [L, L]
    S = softmax(scores, -1)
    V = v_n @ x_b^T                         [DH, L]
    out[b, l, n*DH+e] = sum_m S[l, m] V[e, m]

Sharding: 48 independent (b, n) units over 8 cores.  Core c owns the
batch PAIR bp = c//4 (batches 2bp, 2bp+1) and the head TRIPLE t = c%4
(heads 3t..3t+3): A = q^T k is computed once per head and reused for
both batches.  No collectives; host only slices inputs and
concatenates outputs.

Device math per (head, batch) with all-bf16 matmuls / f32 PSUM:
    A  = q_n^T k_n                [D, D]    (once per head)
    WT = A^T x_b^T                [D, L]
    sT(mj) = xtT-block^T @ WT     [128m, L]  (scores transposed)
    pT(mj) = exp(sT(mj))          bf16       (no max subtraction:
                                              logits are O(1) here)
    R^T[l-block, 0:64] + sums[l] via matmul with vt_aug (V^T plus a
    ones column -> column 64 accumulates sum_m exp) accumulated over mj
    out_block = R^T * (1/sums)    per-partition tensor_scalar
Output per core: out_r [2, L, 192]; host writes out_r[bi] straight
into out[2bp+bi, :, 192t:192t+192].

Scheduling: q0/k0 are loaded first so the PE can start on A(h0) as
early as possible; casts are spread over ACT (q), GpSimd (k) and DVE
(xT/vT) to shorten the critical path; VT(bi) projections are emitted
lazily right before first use; output DMA is issued per l-block as
soon as the last head has written it.
"""

from contextlib import ExitStack

import numpy as np

import concourse.bass as bass
import concourse.tile as tile
from concourse import bacc, mybir
from concourse.bass import ts, ds
from concourse.bass_utils import run_bass_kernel_spmd

B, L, D, H = 4, 1024, 768, 12
DH = D // H          # 64
HPC = 3              # heads per core
BPC = 2              # batches per core
N_CORES = 8
DC = D // 128        # 6 chunks of the contraction/feature dim
LB = L // 128        # 8 l-blocks / m-blocks
DHA = DH + 1         # 65: head slice width in vt_aug (ones column at 64)
F32 = mybir.dt.float32
BF16 = mybir.dt.bfloat16

_COMPILED = None


def _build():
    nc = bacc.Bacc(
        "TRN2",
        target_bir_lowering=False,
        debug=False,
        enable_asserts=False,
        num_devices=N_CORES,
    )
    xT_ext = nc.dram_tensor("xT", [BPC, D, L], F32, kind="ExternalInput").ap()
    q3_ext = nc.dram_tensor("q3", [HPC, D, D], F32, kind="ExternalInput").ap()
    k3_ext = nc.dram_tensor("k3", [HPC, D, D], F32, kind="ExternalInput").ap()
    vT3_ext = nc.dram_tensor("vT3", [D, HPC * DH], F32, kind="ExternalInput").ap()
    out_ext = nc.dram_tensor(
        "out_r", [BPC, L, HPC * DH], F32, kind="ExternalOutput"
    ).ap()

    with tile.TileContext(nc) as tc, ExitStack() as ctx:
        stage_pool = ctx.enter_context(tc.tile_pool(name="stage", bufs=4))
        xt_pool = ctx.enter_context(tc.tile_pool(name="xt", bufs=1))
        vt3_pool = ctx.enter_context(tc.tile_pool(name="vt3", bufs=1))
        vt_pool = ctx.enter_context(tc.tile_pool(name="vt", bufs=1))
        qk_pool = ctx.enter_context(tc.tile_pool(name="qk", bufs=2))
        a_pool = ctx.enter_context(tc.tile_pool(name="a", bufs=1))
        wt_pool = ctx.enter_context(tc.tile_pool(name="wt", bufs=1))
        pt_pool = ctx.enter_context(tc.tile_pool(name="pt", bufs=1))
        soft_pool = ctx.enter_context(tc.tile_pool(name="soft", bufs=2))
        out_pool = ctx.enter_context(tc.tile_pool(name="outp", bufs=1))
        ps_p = ctx.enter_context(tc.tile_pool(name="ps_p", bufs=2, space="PSUM"))
        ps_s = ctx.enter_context(tc.tile_pool(name="ps_s", bufs=2, space="PSUM"))
        ps_r = ctx.enter_context(tc.tile_pool(name="ps_r", bufs=2, space="PSUM"))

        # ---------- loads, critical-path first ----------
        def load_qk(h):
            q_sb, k_sb = [], []
            for i in range(DC):
                stq = stage_pool.tile([128, D], F32, tag="stg_q")
                nc.sync.dma_start(stq[:], q3_ext[h, ts(i, 128), :])
                tq = qk_pool.tile([128, D], BF16, tag=f"q{i}")
                nc.scalar.copy(tq[:], stq[:])
                q_sb.append(tq)
                stk = stage_pool.tile([128, D], F32, tag="stg_k")
                nc.sync.dma_start(stk[:], k3_ext[h, ts(i, 128), :])
                tk = qk_pool.tile([128, D], BF16, tag=f"k{i}")
                nc.gpsimd.tensor_copy(tk[:], stk[:])
                k_sb.append(tk)
            return q_sb, k_sb

        qk = [load_qk(0)]  # head 0 first: unblocks A(h0)

        xt = [[], []]
        for bi in range(BPC):
            for i in range(DC):
                stg = stage_pool.tile([128, L], F32, tag="stg_x")
                nc.sync.dma_start(stg[:], xT_ext[bi, ts(i, 128), :])
                t = xt_pool.tile([128, L], BF16, tag=f"xt{bi}_{i}")
                nc.vector.tensor_copy(t[:], stg[:])
                xt[bi].append(t)

        vt3 = []
        for i in range(DC):
            stg = stage_pool.tile([128, HPC * DH], F32, tag="stg_v")
            nc.sync.dma_start(stg[:], vT3_ext[ts(i, 128), :])
            t = vt3_pool.tile([128, HPC * DH], BF16, tag=f"vt3_{i}")
            nc.vector.tensor_copy(t[:], stg[:])
            vt3.append(t)

        for h in range(1, HPC):
            qk.append(load_qk(h))

        # ---------- lazy VT_aug projection per batch ----------
        vt = [None, None]

        def build_vt(bi):
            tiles = []
            for j in range(LB):
                p = ps_p.tile([128, 512], F32, tag="ps_p")
                for i in range(DC):
                    nc.tensor.matmul(
                        p[:, : HPC * DH],
                        xt[bi][i][:, ts(j, 128)],
                        vt3[i][:],
                        start=(i == 0),
                        stop=(i == DC - 1),
                    )
                t = vt_pool.tile([128, HPC * DHA], BF16, tag=f"vt{bi}_{j}")
                nc.gpsimd.memset(t[:], 1.0)
                t3 = t[:].rearrange("p (h c) -> p h c", h=HPC)
                p3 = p[:, : HPC * DH].rearrange("p (h c) -> p h c", h=HPC)
                nc.vector.tensor_copy(t3[:, :, :DH], p3[:])
                tiles.append(t)
            vt[bi] = tiles

        # out accumulators: per batch, one [128, 192] f32 tile per l-block
        out_sb = [[], []]
        for bi in range(BPC):
            for lb in range(LB):
                ot = out_pool.tile([128, HPC * DH], F32, tag=f"out{bi}_{lb}")
                out_sb[bi].append(ot)

        for h in range(HPC):
            q_sb, k_sb = qk[h]

            # ---- A[d, d'] = sum_c q[c,d] k[c,d']  (once per head) ----
            a_sb = []
            for i in range(DC):
                t = a_pool.tile([128, D], BF16, tag=f"a{i}")
                for n in range(2):
                    p = ps_p.tile([128, 512], F32, tag="ps_p")
                    for j in range(DC):
                        nc.tensor.matmul(
                            p[:, :384],
                            q_sb[j][:, ts(i, 128)],
                            k_sb[j][:, ts(n, 384)],
                            start=(j == 0),
                            stop=(j == DC - 1),
                        )
                    nc.vector.tensor_copy(t[:, ts(n, 384)], p[:, :384])
                a_sb.append(t)

            for bi in range(BPC):
                xtb = xt[bi]
                # ---- WT[d', l] = sum_d A[d,d'] xT[d,l] ----
                wt_sb = []
                for i in range(DC):
                    t = wt_pool.tile([128, L], BF16, tag=f"wt{i}")
                    for n in range(2):
                        p = ps_p.tile([128, 512], F32, tag="ps_p")
                        for j in range(DC):
                            nc.tensor.matmul(
                                p[:],
                                a_sb[j][:, ts(i, 128)],
                                xtb[j][:, ts(n, 512)],
                                start=(j == 0),
                                stop=(j == DC - 1),
                            )
                        nc.vector.tensor_copy(t[:, ts(n, 512)], p[:])
                    wt_sb.append(t)

                if h == 0:
                    build_vt(bi)  # after first WT: PE has a head start

                # ---- scoresT blocks + exp (pipelined over mj) ----
                def scores_t(mj):
                    p = ps_s.tile([128, L], F32, tag="ps_s")
                    for n in range(2):
                        for j in range(DC):
                            nc.tensor.matmul(
                                p[:, ts(n, 512)],
                                xtb[j][:, ts(mj, 128)],
                                wt_sb[j][:, ts(n, 512)],
                                start=(j == 0),
                                stop=(j == DC - 1),
                            )
                    return p

                pt_sb = []
                ps_prev = scores_t(0)
                for mj in range(LB):
                    ps_cur = ps_prev
                    if mj + 1 < LB:
                        ps_prev = scores_t(mj + 1)
                    pt = pt_pool.tile([128, L], BF16, tag=f"pt{mj}")
                    nc.scalar.activation(
                        pt[:], ps_cur[:], mybir.ActivationFunctionType.Exp
                    )
                    pt_sb.append(pt)

                # ---- R^T per l-block + fused sums -> normalize ----
                for lb in range(LB):
                    pr = ps_r.tile([128, DHA], F32, tag="ps_r")
                    for mj in range(LB):
                        nc.tensor.matmul(
                            pr[:],
                            pt_sb[mj][:, ts(lb, 128)],
                            vt[bi][mj][:, ds(DHA * h, DHA)],
                            start=(mj == 0),
                            stop=(mj == LB - 1),
                        )
                    recip = soft_pool.tile([128, 1], F32, tag="recip")
                    nc.vector.reciprocal(recip[:], pr[:, DH : DH + 1])
                    nc.vector.tensor_scalar_mul(
                        out_sb[bi][lb][:, ts(h, DH)], pr[:, :DH], recip[:]
                    )
                    if h == HPC - 1:
                        nc.sync.dma_start(
                            out_ext[bi, ts(lb, 128), :], out_sb[bi][lb][:]
                        )

    nc.compile()
    return nc


def kernel(x, k, q, v):
    global _COMPILED
    if _COMPILED is None:
        _COMPILED = _build()

    x = np.ascontiguousarray(x, dtype=np.float32)
    k = np.ascontiguousarray(k, dtype=np.float32)
    q = np.ascontiguousarray(q, dtype=np.float32)
    v = np.ascontiguousarray(v, dtype=np.float32)

    in_maps = []
    for c in range(N_CORES):
        bp, t = c // 4, c % 4
        hs = slice(HPC * t, HPC * (t + 1))
        in_maps.append(
            {
                "xT": np.ascontiguousarray(
                    x[BPC * bp : BPC * (bp + 1)].transpose(0, 2, 1)
                ),
                "q3": q[hs],
                "k3": k[hs],
                "vT3": np.ascontiguousarray(
                    v[hs].transpose(2, 0, 1).reshape(D, HPC * DH)
                ),
            }
        )

    res = run_bass_kernel_spmd(_COMPILED, in_maps, core_ids=list(range(N_CORES)))

    out = np.empty((B, L, D), np.float32)
    for c in range(N_CORES):
        bp, t = c // 4, c % 4
        for bi in range(BPC):
            out[BPC * bp + bi, :, HPC * DH * t : HPC * DH * (t + 1)] = res.results[
                c
            ]["out_r"][bi]
    return out


if __name__ == "__main__":
    rng = np.random.default_rng(0)
    x = rng.standard_normal((B, L, D)).astype(np.float32)
    k = (rng.random((H, D, D)) / D).astype(np.float32)
    q = (rng.random((H, D, D)) / D).astype(np.float32)
    v = (rng.random((H, DH, D)) / D).astype(np.float32)
    o = kernel(x=x, k=k, q=q, v=v)
    print("out", o.shape, o.dtype)


# revision 10
# speedup vs baseline: 1.9000x; 1.0351x over previous
"""Trainium2 Bass kernel for nn_Attention (B=4, L=1024, D=768, H=12, DH=64).

Reference per (batch b, head n):
    K = k_n @ x_b^T; Q = q_n @ x_b^T        [D, L]
    scores = Q^T K                          [L, L]
    S = softmax(scores, -1)
    V = v_n @ x_b^T                         [DH, L]
    out[b, l, n*DH+e] = sum_m S[l, m] V[e, m]

Sharding: 48 independent (b, n) units over 8 cores.  Core c owns the
batch PAIR bp = c//4 (batches 2bp, 2bp+1) and the head TRIPLE t = c%4
(heads 3t..3t+3): A = q^T k is computed once per head and reused for
both batches.  No collectives; host only slices inputs and
concatenates outputs.

Device math per (head, batch) with all-bf16 matmuls / f32 PSUM:
    A  = q_n^T k_n                [D, D]    (once per head)
    WT = A^T x_b^T                [D, L]
    sT(mj) = xtT-block^T @ WT     [128m, L]  (scores transposed)
    pT(mj) = exp(sT(mj))          bf16       (no max subtraction:
                                              logits are O(1) here)
    R^T[l-block, 0:64] + sums[l] via matmul with vt_aug (V^T plus a
    ones column -> column 64 accumulates sum_m exp) accumulated over mj
    out_block = R^T * (1/sums)    per-partition tensor_scalar
Output per core: out_r [2, L, 192]; host writes out_r[bi] straight
into out[2bp+bi, :, 192t:192t+192].

Scheduling: q0/k0 are loaded first so the PE can start on A(h0) as
early as possible; casts are spread over ACT (q), GpSimd (k) and DVE
(xT/vT) to shorten the critical path; VT(bi) projections are emitted
lazily right before first use; output DMA is issued per l-block as
soon as the last head has written it.
"""

from contextlib import ExitStack

import ml_dtypes
import numpy as np

import concourse.bass as bass
import concourse.tile as tile
from concourse import bacc, mybir
from concourse.bass import ts, ds
from concourse.bass_utils import run_bass_kernel_spmd

B, L, D, H = 4, 1024, 768, 12
DH = D // H          # 64
HPC = 3              # heads per core
BPC = 2              # batches per core
N_CORES = 8
DC = D // 128        # 6 chunks of the contraction/feature dim
LB = L // 128        # 8 l-blocks / m-blocks
DHA = DH + 1         # 65: head slice width in vt_aug (ones column at 64)
F32 = mybir.dt.float32
BF16 = mybir.dt.bfloat16

_COMPILED = None


def _build():
    nc = bacc.Bacc(
        "TRN2",
        target_bir_lowering=False,
        debug=False,
        enable_asserts=False,
        num_devices=N_CORES,
    )
    xT_ext = nc.dram_tensor("xT", [BPC, D, L], BF16, kind="ExternalInput").ap()
    q3_ext = nc.dram_tensor("q3", [HPC, D, D], BF16, kind="ExternalInput").ap()
    k3_ext = nc.dram_tensor("k3", [HPC, D, D], BF16, kind="ExternalInput").ap()
    vT3_ext = nc.dram_tensor("vT3", [D, HPC * DH], BF16, kind="ExternalInput").ap()
    out_ext = nc.dram_tensor(
        "out_r", [BPC, L, HPC * DH], F32, kind="ExternalOutput"
    ).ap()

    with tile.TileContext(nc) as tc, ExitStack() as ctx:
        xt_pool = ctx.enter_context(tc.tile_pool(name="xt", bufs=1))
        vt3_pool = ctx.enter_context(tc.tile_pool(name="vt3", bufs=1))
        vt_pool = ctx.enter_context(tc.tile_pool(name="vt", bufs=1))
        qk_pool = ctx.enter_context(tc.tile_pool(name="qk", bufs=2))
        a_pool = ctx.enter_context(tc.tile_pool(name="a", bufs=1))
        wt_pool = ctx.enter_context(tc.tile_pool(name="wt", bufs=1))
        pt_pool = ctx.enter_context(tc.tile_pool(name="pt", bufs=1))
        soft_pool = ctx.enter_context(tc.tile_pool(name="soft", bufs=2))
        out_pool = ctx.enter_context(tc.tile_pool(name="outp", bufs=1))
        ps_p = ctx.enter_context(tc.tile_pool(name="ps_p", bufs=2, space="PSUM"))
        ps_s = ctx.enter_context(tc.tile_pool(name="ps_s", bufs=2, space="PSUM"))
        ps_r = ctx.enter_context(tc.tile_pool(name="ps_r", bufs=2, space="PSUM"))

        # ---------- loads, critical-path first ----------
        def load_qk(h):
            q_sb, k_sb = [], []
            for i in range(DC):
                tq = qk_pool.tile([128, D], BF16, tag=f"q{i}")
                nc.sync.dma_start(tq[:], q3_ext[h, ts(i, 128), :])
                q_sb.append(tq)
                tk = qk_pool.tile([128, D], BF16, tag=f"k{i}")
                nc.sync.dma_start(tk[:], k3_ext[h, ts(i, 128), :])
                k_sb.append(tk)
            return q_sb, k_sb

        qk = [load_qk(0)]  # head 0 first: unblocks A(h0)

        xt = [[], []]
        for bi in range(BPC):
            for i in range(DC):
                t = xt_pool.tile([128, L], BF16, tag=f"xt{bi}_{i}")
                nc.sync.dma_start(t[:], xT_ext[bi, ts(i, 128), :])
                xt[bi].append(t)

        vt3 = []
        for i in range(DC):
            t = vt3_pool.tile([128, HPC * DH], BF16, tag=f"vt3_{i}")
            nc.sync.dma_start(t[:], vT3_ext[ts(i, 128), :])
            vt3.append(t)

        for h in range(1, HPC):
            qk.append(load_qk(h))

        # ---------- lazy VT_aug projection per batch ----------
        vt = [None, None]

        def build_vt(bi):
            tiles = []
            for j in range(LB):
                p = ps_p.tile([128, 512], F32, tag="ps_p")
                for i in range(DC):
                    nc.tensor.matmul(
                        p[:, : HPC * DH],
                        xt[bi][i][:, ts(j, 128)],
                        vt3[i][:],
                        start=(i == 0),
                        stop=(i == DC - 1),
                    )
                t = vt_pool.tile([128, HPC * DHA], BF16, tag=f"vt{bi}_{j}")
                nc.gpsimd.memset(t[:], 1.0)
                t3 = t[:].rearrange("p (h c) -> p h c", h=HPC)
                p3 = p[:, : HPC * DH].rearrange("p (h c) -> p h c", h=HPC)
                nc.vector.tensor_copy(t3[:, :, :DH], p3[:])
                tiles.append(t)
            vt[bi] = tiles

        # out accumulators: per batch, one [128, 192] f32 tile per l-block
        out_sb = [[], []]
        for bi in range(BPC):
            for lb in range(LB):
                ot = out_pool.tile([128, HPC * DH], F32, tag=f"out{bi}_{lb}")
                out_sb[bi].append(ot)

        for h in range(HPC):
            q_sb, k_sb = qk[h]

            # ---- A[d, d'] = sum_c q[c,d] k[c,d']  (once per head) ----
            a_sb = []
            for i in range(DC):
                t = a_pool.tile([128, D], BF16, tag=f"a{i}")
                for n in range(2):
                    p = ps_p.tile([128, 512], F32, tag="ps_p")
                    for j in range(DC):
                        nc.tensor.matmul(
                            p[:, :384],
                            q_sb[j][:, ts(i, 128)],
                            k_sb[j][:, ts(n, 384)],
                            start=(j == 0),
                            stop=(j == DC - 1),
                        )
                    nc.vector.tensor_copy(t[:, ts(n, 384)], p[:, :384])
                a_sb.append(t)

            for bi in range(BPC):
                xtb = xt[bi]
                # ---- WT[d', l] = sum_d A[d,d'] xT[d,l] ----
                wt_sb = []
                for i in range(DC):
                    t = wt_pool.tile([128, L], BF16, tag=f"wt{i}")
                    for n in range(2):
                        p = ps_p.tile([128, 512], F32, tag="ps_p")
                        for j in range(DC):
                            nc.tensor.matmul(
                                p[:],
                                a_sb[j][:, ts(i, 128)],
                                xtb[j][:, ts(n, 512)],
                                start=(j == 0),
                                stop=(j == DC - 1),
                            )
                        nc.vector.tensor_copy(t[:, ts(n, 512)], p[:])
                    wt_sb.append(t)

                if h == 0:
                    build_vt(bi)  # after first WT: PE has a head start

                # ---- scoresT blocks + exp (pipelined over mj) ----
                def scores_t(mj):
                    p = ps_s.tile([128, L], F32, tag="ps_s")
                    for n in range(2):
                        for j in range(DC):
                            nc.tensor.matmul(
                                p[:, ts(n, 512)],
                                xtb[j][:, ts(mj, 128)],
                                wt_sb[j][:, ts(n, 512)],
                                start=(j == 0),
                                stop=(j == DC - 1),
                            )
                    return p

                pt_sb = []
                ps_prev = scores_t(0)
                for mj in range(LB):
                    ps_cur = ps_prev
                    if mj + 1 < LB:
                        ps_prev = scores_t(mj + 1)
                    pt = pt_pool.tile([128, L], BF16, tag=f"pt{mj}")
                    nc.scalar.activation(
                        pt[:], ps_cur[:], mybir.ActivationFunctionType.Exp
                    )
                    pt_sb.append(pt)

                # ---- R^T per l-block + fused sums -> normalize ----
                for lb in range(LB):
                    pr = ps_r.tile([128, DHA], F32, tag="ps_r")
                    for mj in range(LB):
                        nc.tensor.matmul(
                            pr[:],
                            pt_sb[mj][:, ts(lb, 128)],
                            vt[bi][mj][:, ds(DHA * h, DHA)],
                            start=(mj == 0),
                            stop=(mj == LB - 1),
                        )
                    recip = soft_pool.tile([128, 1], F32, tag="recip")
                    nc.vector.reciprocal(recip[:], pr[:, DH : DH + 1])
                    nc.vector.tensor_scalar_mul(
                        out_sb[bi][lb][:, ts(h, DH)], pr[:, :DH], recip[:]
                    )
                    if h == HPC - 1:
                        nc.sync.dma_start(
                            out_ext[bi, ts(lb, 128), :], out_sb[bi][lb][:]
                        )

    nc.compile()
    return nc


def kernel(x, k, q, v):
    global _COMPILED
    if _COMPILED is None:
        _COMPILED = _build()

    x = np.ascontiguousarray(x, dtype=np.float32)
    k = np.ascontiguousarray(k, dtype=np.float32)
    q = np.ascontiguousarray(q, dtype=np.float32)
    v = np.ascontiguousarray(v, dtype=np.float32)

    bf = ml_dtypes.bfloat16
    xb = x.transpose(0, 2, 1).astype(bf)   # [B, D, L]
    qb = q.astype(bf)
    kb = k.astype(bf)
    vb = v.transpose(2, 0, 1).astype(bf)   # [D, H, DH]
    in_maps = []
    for c in range(N_CORES):
        bp, t = c // 4, c % 4
        hs = slice(HPC * t, HPC * (t + 1))
        in_maps.append(
            {
                "xT": np.ascontiguousarray(xb[BPC * bp : BPC * (bp + 1)]),
                "q3": np.ascontiguousarray(qb[hs]),
                "k3": np.ascontiguousarray(kb[hs]),
                "vT3": np.ascontiguousarray(
                    vb[:, hs].reshape(D, HPC * DH)
                ),
            }
        )

    res = run_bass_kernel_spmd(_COMPILED, in_maps, core_ids=list(range(N_CORES)))

    out = np.empty((B, L, D), np.float32)
    for c in range(N_CORES):
        bp, t = c // 4, c % 4
        for bi in range(BPC):
            out[BPC * bp + bi, :, HPC * DH * t : HPC * DH * (t + 1)] = res.results[
                c
            ]["out_r"][bi]
    return out


if __name__ == "__main__":
    rng = np.random.default_rng(0)
    x = rng.standard_normal((B, L, D)).astype(np.float32)
    k = (rng.random((H, D, D)) / D).astype(np.float32)
    q = (rng.random((H, D, D)) / D).astype(np.float32)
    v = (rng.random((H, DH, D)) / D).astype(np.float32)
    o = kernel(x=x, k=k, q=q, v=v)
    print("out", o.shape, o.dtype)
